# revision 16
# baseline (speedup 1.0000x reference)
"""ConvLSTM cell kernel for Trainium2 (8 NeuronCores).

Sharding: data-parallel over batch B=4 x spatial split of H=64 into 2 halves
(8 shards). The recurrence prevents sharding T. Each core computes its half
with a shrinking row margin (47-t rows at step t) so no cross-core
communication is ever needed: row validity shrinks by 1 per conv step, and
16 margin rows cover all 16 steps. Bottom halves are row-flipped on the host
(x rows flipped + conv kernel dy-flipped) so a single SPMD program serves
all 8 cores.

On-core layout:
  h lives in SBUF as [128, 49, 68] bf16 "HB": partitions 64-127 hold hpad
  (1 zero pad row on top, 2 zero pad cols left, 2 right), partitions 0-63
  hold the same data shifted down one row. A 3x3 conv then needs only 6
  matmul issues per 128-wide oc tile: 3 K=128 issues cover tap pairs
  (dy=1 on 0-63, dy=0 on 64-127) for dx=0..2, and 3 K=64 issues cover dy=2
  from the hpad half. x_t is added in PSUM with an identity matmul issued
  FIRST (start=True) so the step boundary has h-independent PE work.

Gate packing (host-side channel perm): tmp0 = [g(0:64); f(64:128)],
tmp1 = [i(0:64); o(64:128)]. One Tanh ACT with per-partition scale [1, 0.5]
gives [g ; s_f=tanh(z/2)]; one Sigmoid ACT gives [sig_i ; sig_o] (both
functions live in the `sigmoid_and_others` table set -> zero switches).
States c and h are carried UNSCALED on partitions 64-127.

State update (DVE; tensor_tensor runs in 2x packed-bf16 mode,
scalar_tensor_tensor is the only 1x op and the only cross-partition one):
  w1 = s_f + 1            (tensor_scalar, 4x)   = 2f
  u  = w1 * c             (TT, 2x)              = 2 f c
  v  = sig_i * g          (TT, 2x)
  c' = 0.5*u + v          (STT, cross-half add)
  tc = tanh(c')           (ACT)
  h' = sig_o * tc         (TT, 2x, into HB rows 64-127)
The tanh/h'/shifted-copy tail of each 16-row block is software-lagged by
one block (carried across the step boundary) so the Scalar and Vector FIFOs
never ping-pong head-of-line within a block, and the next step's matmuls
start with zero exposed serial tail.
"""

import sys

sys.path.insert(0, "/opt/trn_rl_repo")

import numpy as np
from ml_dtypes import bfloat16

HIDDEN = 64
T_STEPS = 16
B = 4
H = 64
W = 64
OC = 4 * HIDDEN  # 256
ROWS = 48        # per-core x rows (32 owned + 16 margin)
OWN = 32
WP = W + 4       # padded row width 68 (2 left, 2 right; keeps 4B alignment)
CL = 2           # left pad columns
HROWS = ROWS + 1  # hpad rows (1 zero row on top)

_CACHE = {}


def _build_nc():
    from concourse import bacc, mybir
    from concourse.tile import TileContext

    dt = mybir.dt
    Alu = mybir.AluOpType
    Act = mybir.ActivationFunctionType

    nc = bacc.Bacc(None, target_bir_lowering=False)

    x_in = nc.dram_tensor("x", [T_STEPS, 2, 128, ROWS * W], dt.bfloat16,
                          kind="ExternalInput")
    wp_in = nc.dram_tensor("wp", [128, 6 * 128], dt.bfloat16,
                           kind="ExternalInput")
    wb_in = nc.dram_tensor("wb", [128, 2 * 128], dt.bfloat16,
                           kind="ExternalInput")
    ws_in = nc.dram_tensor("ws", [64, 2 * 128], dt.bfloat16,
                           kind="ExternalInput")
    id_in = nc.dram_tensor("ident", [128, 128], dt.bfloat16,
                           kind="ExternalInput")
    sc_in = nc.dram_tensor("scale2", [128, 1], dt.float32,
                           kind="ExternalInput")
    hout = nc.dram_tensor("hout", [T_STEPS, 64, OWN * W], dt.bfloat16,
                          kind="ExternalOutput")

    with TileContext(nc) as tc:
        with (
            tc.tile_pool(name="const", bufs=1) as cpool,
            tc.tile_pool(name="state", bufs=1) as spool,
            tc.tile_pool(name="xload", bufs=3) as xpool,
            tc.tile_pool(name="work", bufs=3) as wpool,
            tc.tile_pool(name="ps", bufs=2, space="PSUM") as psp,
        ):
            wp_sb = cpool.tile([128, 6 * 128], dt.bfloat16, tag="wp")
            wb_sb = cpool.tile([128, 2 * 128], dt.bfloat16, tag="wb")
            # ws lives on partitions 64-127 to match the hpad half of HB
            # (matmul requires lhsT and rhs at the same base partition)
            ws_sb = cpool.tile([128, 2 * 128], dt.bfloat16, tag="ws")
            id_sb = cpool.tile([128, 128], dt.bfloat16, tag="id")
            sc_sb = cpool.tile([128, 1], dt.float32, tag="sc")
            nc.sync.dma_start(out=wp_sb[:], in_=wp_in[:])
            nc.sync.dma_start(out=wb_sb[:], in_=wb_in[:])
            nc.sync.dma_start(out=ws_sb[64:128, :], in_=ws_in[:])
            nc.sync.dma_start(out=id_sb[:], in_=id_in[:])
            nc.sync.dma_start(out=sc_sb[:], in_=sc_in[:])

            # h buffers (ping-pong across steps) and cell state c.
            # bb: the dy=2 companion of hb -- partitions 64-127 mirror hpad,
            # partitions 0-63 hold hpad shifted LEFT one column, so one
            # K=128 matmul covers the (dy2,dx0)+(dy2,dx1) tap pair.
            hb = [
                spool.tile([128, HROWS, WP], dt.bfloat16, tag="hb0", name="hb0"),
                spool.tile([128, HROWS, WP], dt.bfloat16, tag="hb1", name="hb1"),
            ]
            bb = [
                spool.tile([128, HROWS, WP], dt.bfloat16, tag="bb0", name="bb0"),
                spool.tile([128, HROWS, WP], dt.bfloat16, tag="bb1", name="bb1"),
            ]
            cs = spool.tile([128, ROWS * W], dt.bfloat16, tag="cs")
            nc.vector.memset(hb[0][:], 0.0)
            nc.vector.memset(hb[1][:], 0.0)
            nc.vector.memset(bb[0][:], 0.0)
            nc.vector.memset(bb[1][:], 0.0)
            nc.vector.memset(cs[:], 0.0)

            def emit_tail(pend):
                """Lagged block tail: tc = tanh(c'); h' = sig_o*tc; copies."""
                p_hbw, p_bbw, y0, rows, Nb, sio, tch, last_t = pend
                cseg = cs[64:128, y0 * W: y0 * W + Nb]
                nc.scalar.activation(tch[64:128, :Nb], cseg, Act.Tanh,
                                     scale=1.0)
                nc.vector.tensor_tensor(
                    p_hbw[64:128, 1 + y0: 1 + y0 + rows, CL: CL + W],
                    sio[64:128, :Nb], tch[64:128, :Nb], Alu.mult)
                if not last_t:
                    # shifted copy for next step's K=128 pairs: lo[r] = hi[r+1]
                    nc.vector.tensor_copy(
                        p_hbw[0:64, y0: y0 + rows, :],
                        p_hbw[64:128, y0 + 1: y0 + 1 + rows, :])
                    # bb companion for the dy=2 (dx0,dx1) pair: partitions
                    # 0-63 = hpad shifted left 1 col, 64-127 = shifted left
                    # 2 cols. Flat contiguous copies on the DMA engines
                    # (SBUF->SBUF), off the compute engines' critical path;
                    # the wrap elements land in hpad's zero pad columns.
                    base = (1 + y0) * WP
                    ln = rows * WP
                    hflat = p_hbw[64:128].rearrange("p a b -> p (a b)")
                    blo = p_bbw[0:64].rearrange("p a b -> p (a b)")
                    bhi = p_bbw[64:128].rearrange("p a b -> p (a b)")
                    # issue from the idle GpSimd queue: these waits must not
                    # block the Sync queue's x-prefetch DMAs
                    nc.gpsimd.dma_start(out=blo[:, base: base + ln],
                                        in_=hflat[:, base + 1: base + 1 + ln])
                    nc.gpsimd.dma_start(out=bhi[:, base: base + ln],
                                        in_=hflat[:, base + 2: base + 2 + ln])

            pend = None      # one-block-lagged tail state
            pend_out = None  # deferred hout DMA: (t, hbw)
            for t in range(T_STEPS):
                R = 47 - t  # output rows this step
                hbr = hb[t % 2]
                hbw = hb[(t + 1) % 2]
                bbr = bb[t % 2]
                bbw = bb[(t + 1) % 2]

                xt = []
                for half in range(2):
                    xti = xpool.tile([128, ROWS * W], dt.bfloat16,
                                     tag=f"x{half}", name=f"x{half}")
                    nc.sync.dma_start(out=xti[:, : R * W],
                                      in_=x_in[t, half][:, : R * W])
                    xt.append(xti)

                nblk = (R + 15) // 16
                for bi in range(nblk):
                    y0 = bi * 16
                    rows = min(16, R - y0)
                    Nb = rows * W

                    tmp = [psp.tile([128, 1024], dt.float32, tag="tmp0",
                                    name="tmp0"),
                           psp.tile([128, 1024], dt.float32, tag="tmp1",
                                    name="tmp1")]
                    nsub = (rows + 7) // 8
                    # x injection first: h-independent PE work at block entry
                    for tau in range(2):
                        for sub in range(nsub):
                            yy = y0 + sub * 8
                            sr = min(8, rows - sub * 8)
                            n = sr * W
                            nc.tensor.matmul(
                                tmp[tau][:, sub * 512: sub * 512 + n],
                                lhsT=id_sb[:],
                                rhs=xt[tau][:, yy * W: yy * W + n],
                                start=True, stop=(t == 0))
                    if t > 0:
                        for tau in range(2):
                            for sub in range(nsub):
                                yy = y0 + sub * 8
                                sr = min(8, rows - sub * 8)
                                n = sr * W
                                out_ap = tmp[tau][:, sub * 512: sub * 512 + n]
                                for d in range(3):
                                    nc.tensor.matmul(
                                        out_ap,
                                        lhsT=wp_sb[:, (tau * 3 + d) * 128:
                                                   (tau * 3 + d + 1) * 128],
                                        rhs=hbr[:, yy: yy + sr,
                                                d + 1: d + 1 + W],
                                        start=False, stop=False)
                                # dy=2: (dx0,dx1) pair via the bb companion
                                nc.tensor.matmul(
                                    out_ap,
                                    lhsT=wb_sb[:, tau * 128:
                                               (tau + 1) * 128],
                                    rhs=bbr[:, yy + 2: yy + 2 + sr,
                                            0: W],
                                    start=False, stop=False)
                                # dy=2, dx2 single K=64 tap from hpad
                                nc.tensor.matmul(
                                    out_ap,
                                    lhsT=ws_sb[64:128, tau * 128:
                                               (tau + 1) * 128],
                                    rhs=hbr[64:128, yy + 2: yy + 2 + sr,
                                            3: 3 + W],
                                    start=False, stop=True)

                    # lagged tail from the previous block (possibly the last
                    # block of the previous step), then any deferred hout DMA
                    if pend is not None:
                        emit_tail(pend)
                        pend = None
                    if pend_out is not None:
                        pt, p_hbw = pend_out
                        nc.scalar.dma_start(
                            out=hout[pt],
                            in_=p_hbw[64:128, 1: 1 + OWN, CL: CL + W])
                        pend_out = None

                    # gates: tile0 = [g ; s_f=tanh(z/2)] (one Tanh act,
                    # per-partition scale), tile1 = [sig_i ; sig_o]
                    sgf = wpool.tile([128, 1024], dt.bfloat16, tag="sgf")
                    sio = wpool.tile([128, 1024], dt.bfloat16, tag="sio")
                    nc.scalar.activation(sgf[:, :Nb], tmp[0][:, :Nb],
                                         Act.Tanh, scale=sc_sb[:])
                    nc.scalar.activation(sio[:, :Nb], tmp[1][:, :Nb],
                                         Act.Sigmoid, scale=1.0)

                    # state update: w1 = 0.5*s_f + 0.5 (= f); u = f*c;
                    # v = sig_i*g (cross-half STT: inputs on 0-63, out on
                    # 64-127 -- engine output partition remap, inputs must
                    # match); c' = u + v
                    u = wpool.tile([128, 1024], dt.bfloat16, tag="u")
                    v = wpool.tile([128, 1024], dt.bfloat16, tag="v")
                    w1 = wpool.tile([128, 1024], dt.bfloat16, tag="w1")
                    tch = wpool.tile([128, 1024], dt.bfloat16, tag="tch")
                    cseg = cs[64:128, y0 * W: y0 * W + Nb]
                    nc.vector.tensor_scalar(
                        w1[64:128, :Nb], sgf[64:128, :Nb], 0.5, 0.5,
                        Alu.mult, Alu.add)
                    nc.vector.tensor_tensor(
                        u[64:128, :Nb], w1[64:128, :Nb], cseg, Alu.mult)
                    nc.vector.scalar_tensor_tensor(
                        v[64:128, :Nb], sio[0:64, :Nb], 1.0, sgf[0:64, :Nb],
                        Alu.mult, Alu.mult)
                    nc.vector.tensor_tensor(
                        cseg, u[64:128, :Nb], v[64:128, :Nb], Alu.add)

                    pend = (hbw, bbw, y0, rows, Nb, sio, tch, t == T_STEPS - 1)

                pend_out = (t, hbw)

            # flush the final block's tail and the last output store
            emit_tail(pend)
            pt, p_hbw = pend_out
            nc.scalar.dma_start(out=hout[pt],
                                in_=p_hbw[64:128, 1: 1 + OWN, CL: CL + W])

    nc.finalize()
    return nc


def _prep_inputs(x, w_h2h):
    """Build per-core input maps. Cores: core = b*2 + half."""
    # gate order in PSUM tiles: tile0 = [g(0:64); f(64:128)],
    # tile1 = [i(0:64); o(64:128)]; reference channel order is [i, f, o, g]
    perm = np.concatenate([np.arange(192, 256), np.arange(64, 128),
                           np.arange(0, 64), np.arange(128, 192)])
    w_eff = w_h2h.astype(np.float32)[perm]  # [256, 64, 3, 3]

    def pack_w(weff):
        # wp lhsT rows: 0-63 multiply the row-shifted h copy (tap dy=1),
        # 64-127 multiply hpad (tap dy=0). wb covers dy=2 x (dx0, dx1) via
        # the bb companion (lo = hpad shifted left 1 col -> tap dx1);
        # ws covers the single remaining (dy2, dx2) tap.
        wp = np.zeros((128, 2, 3, 128), np.float32)
        wb = np.zeros((128, 2, 128), np.float32)
        ws = np.zeros((64, 2, 128), np.float32)
        for tau in range(2):
            blk = weff[tau * 128: (tau + 1) * 128]  # [128oc, 64ic, 3, 3]
            for d in range(3):
                wp[0:64, tau, d, :] = blk[:, :, 1, d].T
                wp[64:128, tau, d, :] = blk[:, :, 0, d].T
            wb[0:64, tau, :] = blk[:, :, 2, 0].T
            wb[64:128, tau, :] = blk[:, :, 2, 1].T
            ws[:, tau, :] = blk[:, :, 2, 2].T
        return (wp.reshape(128, 6 * 128).astype(bfloat16),
                wb.reshape(128, 2 * 128).astype(bfloat16),
                ws.reshape(64, 2 * 128).astype(bfloat16))

    wp_top, wb_top, ws_top = pack_w(w_eff)
    wp_bot, wb_bot, ws_bot = pack_w(w_eff[:, :, ::-1, :])
    ident = np.eye(128, dtype=np.float32).astype(bfloat16)
    scale2 = np.concatenate([np.ones((64, 1), np.float32),
                             np.full((64, 1), 0.5, np.float32)])

    xp = x[:, :, perm]  # [T, B, 256, H, W] permuted channels
    in_maps = []
    for b in range(B):
        for half in range(2):
            if half == 0:
                xs = xp[:, b, :, 0:ROWS, :]
            else:
                xs = xp[:, b, :, H - ROWS:, :][:, :, ::-1, :]
            xs = np.ascontiguousarray(xs).astype(bfloat16)
            xs = xs.reshape(T_STEPS, 2, 128, ROWS * W)
            in_maps.append({
                "x": xs,
                "wp": wp_top if half == 0 else wp_bot,
                "wb": wb_top if half == 0 else wb_bot,
                "ws": ws_top if half == 0 else ws_bot,
                "ident": ident,
                "scale2": scale2,
            })
    return in_maps


def kernel(x, w_h2h):
    from concourse import bass_utils

    if "nc" not in _CACHE:
        _CACHE["nc"] = _build_nc()
    nc = _CACHE["nc"]

    in_maps = _prep_inputs(np.asarray(x), np.asarray(w_h2h))
    res = bass_utils.run_bass_kernel_spmd(nc, in_maps,
                                          core_ids=list(range(8)))
    _CACHE["last_results"] = res

    out = np.zeros((T_STEPS, B, HIDDEN, H, W), np.float32)
    for b in range(B):
        for half in range(2):
            core = b * 2 + half
            hs = res.results[core]["hout"].astype(np.float32)
            hs = hs.reshape(T_STEPS, HIDDEN, OWN, W)
            if half == 0:
                out[:, b, :, 0:OWN, :] = hs
            else:
                out[:, b, :, OWN:, :] = hs[:, :, ::-1, :]
    return out


# revision 18
# speedup vs baseline: 1.0259x; 1.0259x over previous
"""ConvLSTM cell kernel for Trainium2 (8 NeuronCores).

Sharding: data-parallel over batch B=4 x spatial split of H=64 into 2 halves
(8 shards). The recurrence prevents sharding T. Each core computes its half
with a shrinking row margin (47-t rows at step t) so no cross-core
communication is ever needed: row validity shrinks by 1 per conv step, and
16 margin rows cover all 16 steps. Bottom halves are row-flipped on the host
(x rows flipped + conv kernel dy-flipped) so a single SPMD program serves
all 8 cores.

On-core layout:
  h lives in SBUF as [128, 49, 68] bf16 "HB": partitions 64-127 hold hpad
  (1 zero pad row on top, 2 zero pad cols left, 2 right), partitions 0-63
  hold the same data shifted down one row. A 3x3 conv then needs only 6
  matmul issues per 128-wide oc tile: 3 K=128 issues cover tap pairs
  (dy=1 on 0-63, dy=0 on 64-127) for dx=0..2, and 3 K=64 issues cover dy=2
  from the hpad half. x_t is added in PSUM with an identity matmul issued
  FIRST (start=True) so the step boundary has h-independent PE work.

Gate packing (host-side channel perm): tmp0 = [g(0:64); f(64:128)],
tmp1 = [i(0:64); o(64:128)]. One Tanh ACT with per-partition scale [1, 0.5]
gives [g ; s_f=tanh(z/2)]; one Sigmoid ACT gives [sig_i ; sig_o] (both
functions live in the `sigmoid_and_others` table set -> zero switches).
States c and h are carried UNSCALED on partitions 64-127.

State update (DVE; tensor_tensor runs in 2x packed-bf16 mode,
scalar_tensor_tensor is the only 1x op and the only cross-partition one):
  w1 = s_f + 1            (tensor_scalar, 4x)   = 2f
  u  = w1 * c             (TT, 2x)              = 2 f c
  v  = sig_i * g          (TT, 2x)
  c' = 0.5*u + v          (STT, cross-half add)
  tc = tanh(c')           (ACT)
  h' = sig_o * tc         (TT, 2x, into HB rows 64-127)
The tanh/h'/shifted-copy tail of each 16-row block is software-lagged by
one block (carried across the step boundary) so the Scalar and Vector FIFOs
never ping-pong head-of-line within a block, and the next step's matmuls
start with zero exposed serial tail.
"""

import sys

sys.path.insert(0, "/opt/trn_rl_repo")

import numpy as np
from ml_dtypes import bfloat16

HIDDEN = 64
T_STEPS = 16
B = 4
H = 64
W = 64
OC = 4 * HIDDEN  # 256
ROWS = 48        # per-core x rows (32 owned + 16 margin)
OWN = 32
WP = W + 4       # padded row width 68 (2 left, 2 right; keeps 4B alignment)
CL = 2           # left pad columns
HROWS = ROWS + 1  # hpad rows (1 zero row on top)

_CACHE = {}


def _build_nc():
    from concourse import bacc, mybir
    from concourse.tile import TileContext

    dt = mybir.dt
    Alu = mybir.AluOpType
    Act = mybir.ActivationFunctionType

    nc = bacc.Bacc(None, target_bir_lowering=False)

    x_in = nc.dram_tensor("x", [T_STEPS, 2, 128, ROWS * W], dt.bfloat16,
                          kind="ExternalInput")
    wp_in = nc.dram_tensor("wp", [128, 6 * 128], dt.bfloat16,
                           kind="ExternalInput")
    wb_in = nc.dram_tensor("wb", [128, 2 * 128], dt.bfloat16,
                           kind="ExternalInput")
    ws_in = nc.dram_tensor("ws", [64, 2 * 128], dt.bfloat16,
                           kind="ExternalInput")
    id_in = nc.dram_tensor("ident", [128, 128], dt.bfloat16,
                           kind="ExternalInput")
    sc_in = nc.dram_tensor("scale2", [128, 1], dt.float32,
                           kind="ExternalInput")
    hout = nc.dram_tensor("hout", [T_STEPS, 64, OWN * W], dt.bfloat16,
                          kind="ExternalOutput")

    with TileContext(nc) as tc:
        with (
            tc.tile_pool(name="const", bufs=1) as cpool,
            tc.tile_pool(name="state", bufs=1) as spool,
            tc.tile_pool(name="xload", bufs=3) as xpool,
            tc.tile_pool(name="work", bufs=3) as wpool,
            tc.tile_pool(name="ps", bufs=2, space="PSUM") as psp,
        ):
            wp_sb = cpool.tile([128, 6 * 128], dt.bfloat16, tag="wp")
            wb_sb = cpool.tile([128, 2 * 128], dt.bfloat16, tag="wb")
            # ws lives on partitions 64-127 to match the hpad half of HB
            # (matmul requires lhsT and rhs at the same base partition)
            ws_sb = cpool.tile([128, 2 * 128], dt.bfloat16, tag="ws")
            id_sb = cpool.tile([128, 128], dt.bfloat16, tag="id")
            sc_sb = cpool.tile([128, 1], dt.float32, tag="sc")
            nc.sync.dma_start(out=wp_sb[:], in_=wp_in[:])
            nc.sync.dma_start(out=wb_sb[:], in_=wb_in[:])
            nc.sync.dma_start(out=ws_sb[64:128, :], in_=ws_in[:])
            nc.sync.dma_start(out=id_sb[:], in_=id_in[:])
            nc.sync.dma_start(out=sc_sb[:], in_=sc_in[:])

            # h buffers (ping-pong across steps) and cell state c.
            # bb: the dy=2 companion of hb -- partitions 64-127 mirror hpad,
            # partitions 0-63 hold hpad shifted LEFT one column, so one
            # K=128 matmul covers the (dy2,dx0)+(dy2,dx1) tap pair.
            hb = [
                spool.tile([128, HROWS, WP], dt.bfloat16, tag="hb0", name="hb0"),
                spool.tile([128, HROWS, WP], dt.bfloat16, tag="hb1", name="hb1"),
            ]
            bb = [
                spool.tile([128, HROWS, WP], dt.bfloat16, tag="bb0", name="bb0"),
                spool.tile([128, HROWS, WP], dt.bfloat16, tag="bb1", name="bb1"),
            ]
            cs = spool.tile([128, ROWS * W], dt.bfloat16, tag="cs")
            nc.vector.memset(hb[0][:], 0.0)
            nc.vector.memset(hb[1][:], 0.0)
            nc.vector.memset(bb[0][:], 0.0)
            nc.vector.memset(bb[1][:], 0.0)
            nc.vector.memset(cs[:], 0.0)

            def emit_tail(pend):
                """Lagged block tail: tc = tanh(c'); h' = sig_o*tc; copies."""
                p_hbw, p_bbw, y0, rows, Nb, sio, tch, last_t, step_last = pend
                cseg = cs[64:128, y0 * W: y0 * W + Nb]
                nc.scalar.activation(tch[64:128, :Nb], cseg, Act.Tanh,
                                     scale=1.0)
                nc.vector.tensor_tensor(
                    p_hbw[64:128, 1 + y0: 1 + y0 + rows, CL: CL + W],
                    sio[64:128, :Nb], tch[64:128, :Nb], Alu.mult)
                if not last_t:
                    # shifted copy for next step's K=128 pairs: lo[r] = hi[r+1]
                    nc.vector.tensor_copy(
                        p_hbw[0:64, y0: y0 + rows, :],
                        p_hbw[64:128, y0 + 1: y0 + 1 + rows, :])
                    # bb companion for the dy=2 (dx0,dx1) pair: partitions
                    # 0-63 = hpad shifted left 1 col, 64-127 = shifted left
                    # 2 cols. Flat contiguous copies; the wrap elements land
                    # in hpad's zero pad columns. Mid-step blocks go via
                    # SBUF->SBUF DMA (a full step of slack, off the compute
                    # engines); the step's LAST block is needed ~one block
                    # after the boundary, so DMA latency (~3.5us) would stall
                    # the PE -- do that one on DVE right behind the h-write.
                    base = (1 + y0) * WP
                    ln = rows * WP
                    hflat = p_hbw[64:128].rearrange("p a b -> p (a b)")
                    blo = p_bbw[0:64].rearrange("p a b -> p (a b)")
                    bhi = p_bbw[64:128].rearrange("p a b -> p (a b)")
                    if step_last:
                        nc.vector.tensor_copy(
                            blo[:, base: base + ln],
                            hflat[:, base + 1: base + 1 + ln])
                        nc.vector.tensor_copy(
                            bhi[:, base: base + ln],
                            hflat[:, base + 2: base + 2 + ln])
                    else:
                        nc.gpsimd.dma_start(
                            out=blo[:, base: base + ln],
                            in_=hflat[:, base + 1: base + 1 + ln])
                        nc.gpsimd.dma_start(
                            out=bhi[:, base: base + ln],
                            in_=hflat[:, base + 2: base + 2 + ln])

            pend = None      # one-block-lagged tail state
            pend_out = None  # deferred hout DMA: (t, hbw)
            for t in range(T_STEPS):
                R = 47 - t  # output rows this step
                hbr = hb[t % 2]
                hbw = hb[(t + 1) % 2]
                bbr = bb[t % 2]
                bbw = bb[(t + 1) % 2]

                xt = []
                for half in range(2):
                    xti = xpool.tile([128, ROWS * W], dt.bfloat16,
                                     tag=f"x{half}", name=f"x{half}")
                    nc.sync.dma_start(out=xti[:, : R * W],
                                      in_=x_in[t, half][:, : R * W])
                    xt.append(xti)

                nblk = (R + 15) // 16
                for bi in range(nblk):
                    y0 = bi * 16
                    rows = min(16, R - y0)
                    Nb = rows * W

                    tmp = [psp.tile([128, 1024], dt.float32, tag="tmp0",
                                    name="tmp0"),
                           psp.tile([128, 1024], dt.float32, tag="tmp1",
                                    name="tmp1")]
                    nsub = (rows + 7) // 8
                    # x injection first: h-independent PE work at block entry
                    for tau in range(2):
                        for sub in range(nsub):
                            yy = y0 + sub * 8
                            sr = min(8, rows - sub * 8)
                            n = sr * W
                            nc.tensor.matmul(
                                tmp[tau][:, sub * 512: sub * 512 + n],
                                lhsT=id_sb[:],
                                rhs=xt[tau][:, yy * W: yy * W + n],
                                start=True, stop=(t == 0))
                    if t > 0:
                        for tau in range(2):
                            for sub in range(nsub):
                                yy = y0 + sub * 8
                                sr = min(8, rows - sub * 8)
                                n = sr * W
                                out_ap = tmp[tau][:, sub * 512: sub * 512 + n]
                                for d in range(3):
                                    nc.tensor.matmul(
                                        out_ap,
                                        lhsT=wp_sb[:, (tau * 3 + d) * 128:
                                                   (tau * 3 + d + 1) * 128],
                                        rhs=hbr[:, yy: yy + sr,
                                                d + 1: d + 1 + W],
                                        start=False, stop=False)
                                # dy=2, dx2 single K=64 tap from hpad
                                nc.tensor.matmul(
                                    out_ap,
                                    lhsT=ws_sb[64:128, tau * 128:
                                               (tau + 1) * 128],
                                    rhs=hbr[64:128, yy + 2: yy + 2 + sr,
                                            3: 3 + W],
                                    start=False, stop=False)
                                # dy=2: (dx0,dx1) pair via the bb companion
                                # (last: its bb copies have the least slack)
                                nc.tensor.matmul(
                                    out_ap,
                                    lhsT=wb_sb[:, tau * 128:
                                               (tau + 1) * 128],
                                    rhs=bbr[:, yy + 2: yy + 2 + sr,
                                            0: W],
                                    start=False, stop=True)

                    # lagged tail from the previous block (possibly the last
                    # block of the previous step), then any deferred hout DMA
                    if pend is not None:
                        emit_tail(pend)
                        pend = None
                    if pend_out is not None:
                        pt, p_hbw = pend_out
                        nc.scalar.dma_start(
                            out=hout[pt],
                            in_=p_hbw[64:128, 1: 1 + OWN, CL: CL + W])
                        pend_out = None

                    # gates: tile0 = [g ; s_f=tanh(z/2)] (one Tanh act,
                    # per-partition scale), tile1 = [sig_i ; sig_o]
                    sgf = wpool.tile([128, 1024], dt.bfloat16, tag="sgf")
                    sio = wpool.tile([128, 1024], dt.bfloat16, tag="sio")
                    nc.scalar.activation(sgf[:, :Nb], tmp[0][:, :Nb],
                                         Act.Tanh, scale=sc_sb[:])
                    nc.scalar.activation(sio[:, :Nb], tmp[1][:, :Nb],
                                         Act.Sigmoid, scale=1.0)

                    # state update: w1 = 0.5*s_f + 0.5 (= f); u = f*c;
                    # v = sig_i*g (cross-half STT: inputs on 0-63, out on
                    # 64-127 -- engine output partition remap, inputs must
                    # match); c' = u + v
                    u = wpool.tile([128, 1024], dt.bfloat16, tag="u")
                    v = wpool.tile([128, 1024], dt.bfloat16, tag="v")
                    w1 = wpool.tile([128, 1024], dt.bfloat16, tag="w1")
                    tch = wpool.tile([128, 1024], dt.bfloat16, tag="tch")
                    cseg = cs[64:128, y0 * W: y0 * W + Nb]
                    nc.vector.tensor_scalar(
                        w1[64:128, :Nb], sgf[64:128, :Nb], 0.5, 0.5,
                        Alu.mult, Alu.add)
                    nc.vector.tensor_tensor(
                        u[64:128, :Nb], w1[64:128, :Nb], cseg, Alu.mult)
                    nc.vector.scalar_tensor_tensor(
                        v[64:128, :Nb], sio[0:64, :Nb], 1.0, sgf[0:64, :Nb],
                        Alu.mult, Alu.mult)
                    nc.vector.tensor_tensor(
                        cseg, u[64:128, :Nb], v[64:128, :Nb], Alu.add)

                    pend = (hbw, bbw, y0, rows, Nb, sio, tch, t == T_STEPS - 1,
                            bi == nblk - 1)

                pend_out = (t, hbw)

            # flush the final block's tail and the last output store
            emit_tail(pend)
            pt, p_hbw = pend_out
            nc.scalar.dma_start(out=hout[pt],
                                in_=p_hbw[64:128, 1: 1 + OWN, CL: CL + W])

    nc.finalize()
    return nc


def _prep_inputs(x, w_h2h):
    """Build per-core input maps. Cores: core = b*2 + half."""
    # gate order in PSUM tiles: tile0 = [g(0:64); f(64:128)],
    # tile1 = [i(0:64); o(64:128)]; reference channel order is [i, f, o, g]
    perm = np.concatenate([np.arange(192, 256), np.arange(64, 128),
                           np.arange(0, 64), np.arange(128, 192)])
    w_eff = w_h2h.astype(np.float32)[perm]  # [256, 64, 3, 3]

    def pack_w(weff):
        # wp lhsT rows: 0-63 multiply the row-shifted h copy (tap dy=1),
        # 64-127 multiply hpad (tap dy=0). wb covers dy=2 x (dx0, dx1) via
        # the bb companion (lo = hpad shifted left 1 col -> tap dx1);
        # ws covers the single remaining (dy2, dx2) tap.
        wp = np.zeros((128, 2, 3, 128), np.float32)
        wb = np.zeros((128, 2, 128), np.float32)
        ws = np.zeros((64, 2, 128), np.float32)
        for tau in range(2):
            blk = weff[tau * 128: (tau + 1) * 128]  # [128oc, 64ic, 3, 3]
            for d in range(3):
                wp[0:64, tau, d, :] = blk[:, :, 1, d].T
                wp[64:128, tau, d, :] = blk[:, :, 0, d].T
            wb[0:64, tau, :] = blk[:, :, 2, 0].T
            wb[64:128, tau, :] = blk[:, :, 2, 1].T
            ws[:, tau, :] = blk[:, :, 2, 2].T
        return (wp.reshape(128, 6 * 128).astype(bfloat16),
                wb.reshape(128, 2 * 128).astype(bfloat16),
                ws.reshape(64, 2 * 128).astype(bfloat16))

    wp_top, wb_top, ws_top = pack_w(w_eff)
    wp_bot, wb_bot, ws_bot = pack_w(w_eff[:, :, ::-1, :])
    ident = np.eye(128, dtype=np.float32).astype(bfloat16)
    scale2 = np.concatenate([np.ones((64, 1), np.float32),
                             np.full((64, 1), 0.5, np.float32)])

    xp = x[:, :, perm]  # [T, B, 256, H, W] permuted channels
    in_maps = []
    for b in range(B):
        for half in range(2):
            if half == 0:
                xs = xp[:, b, :, 0:ROWS, :]
            else:
                xs = xp[:, b, :, H - ROWS:, :][:, :, ::-1, :]
            xs = np.ascontiguousarray(xs).astype(bfloat16)
            xs = xs.reshape(T_STEPS, 2, 128, ROWS * W)
            in_maps.append({
                "x": xs,
                "wp": wp_top if half == 0 else wp_bot,
                "wb": wb_top if half == 0 else wb_bot,
                "ws": ws_top if half == 0 else ws_bot,
                "ident": ident,
                "scale2": scale2,
            })
    return in_maps


def kernel(x, w_h2h):
    from concourse import bass_utils

    if "nc" not in _CACHE:
        _CACHE["nc"] = _build_nc()
    nc = _CACHE["nc"]

    in_maps = _prep_inputs(np.asarray(x), np.asarray(w_h2h))
    res = bass_utils.run_bass_kernel_spmd(nc, in_maps,
                                          core_ids=list(range(8)))
    _CACHE["last_results"] = res

    out = np.zeros((T_STEPS, B, HIDDEN, H, W), np.float32)
    for b in range(B):
        for half in range(2):
            core = b * 2 + half
            hs = res.results[core]["hout"].astype(np.float32)
            hs = hs.reshape(T_STEPS, HIDDEN, OWN, W)
            if half == 0:
                out[:, b, :, 0:OWN, :] = hs
            else:
                out[:, b, :, OWN:, :] = hs[:, :, ::-1, :]
    return out


# revision 19
# speedup vs baseline: 1.1608x; 1.1315x over previous
"""ConvLSTM cell kernel for Trainium2 (8 NeuronCores).

Sharding: data-parallel over batch B=4 x spatial split of H=64 into 2 halves
(8 shards). The recurrence prevents sharding T. Each core computes its half
with a shrinking row margin (47-t rows at step t) so no cross-core
communication is ever needed: row validity shrinks by 1 per conv step, and
16 margin rows cover all 16 steps. Bottom halves are row-flipped on the host
(x rows flipped + conv kernel dy-flipped) so a single SPMD program serves
all 8 cores.

On-core layout:
  h lives in SBUF as [128, 49, 68] bf16 "HB": partitions 64-127 hold hpad
  (1 zero pad row on top, 2 zero pad cols left, 2 right), partitions 0-63
  hold the same data shifted down one row. A 3x3 conv then needs only 6
  matmul issues per 128-wide oc tile: 3 K=128 issues cover tap pairs
  (dy=1 on 0-63, dy=0 on 64-127) for dx=0..2, and 3 K=64 issues cover dy=2
  from the hpad half. x_t is added in PSUM with an identity matmul issued
  FIRST (start=True) so the step boundary has h-independent PE work.

Gate packing (host-side channel perm): tmp0 = [g(0:64); f(64:128)],
tmp1 = [i(0:64); o(64:128)]. One Tanh ACT with per-partition scale [1, 0.5]
gives [g ; s_f=tanh(z/2)]; one Sigmoid ACT gives [sig_i ; sig_o] (both
functions live in the `sigmoid_and_others` table set -> zero switches).
States c and h are carried UNSCALED on partitions 64-127.

State update (DVE; tensor_tensor runs in 2x packed-bf16 mode,
scalar_tensor_tensor is the only 1x op and the only cross-partition one):
  w1 = s_f + 1            (tensor_scalar, 4x)   = 2f
  u  = w1 * c             (TT, 2x)              = 2 f c
  v  = sig_i * g          (TT, 2x)
  c' = 0.5*u + v          (STT, cross-half add)
  tc = tanh(c')           (ACT)
  h' = sig_o * tc         (TT, 2x, into HB rows 64-127)
The tanh/h'/shifted-copy tail of each 16-row block is software-lagged by
one block (carried across the step boundary) so the Scalar and Vector FIFOs
never ping-pong head-of-line within a block, and the next step's matmuls
start with zero exposed serial tail.
"""

import sys

sys.path.insert(0, "/opt/trn_rl_repo")

import numpy as np
from ml_dtypes import bfloat16

HIDDEN = 64
T_STEPS = 16
B = 4
H = 64
W = 64
OC = 4 * HIDDEN  # 256
ROWS = 48        # per-core x rows (32 owned + 16 margin)
OWN = 32
WP = W + 4       # padded row width 68 (2 left, 2 right; keeps 4B alignment)
CL = 2           # left pad columns
HROWS = ROWS + 1  # hpad rows (1 zero row on top)

_CACHE = {}


def _build_nc():
    from concourse import bacc, mybir
    from concourse.tile import TileContext

    dt = mybir.dt
    Alu = mybir.AluOpType
    Act = mybir.ActivationFunctionType

    nc = bacc.Bacc(None, target_bir_lowering=False)

    x_in = nc.dram_tensor("x", [T_STEPS, 2, 128, ROWS * W], dt.bfloat16,
                          kind="ExternalInput")
    wp_in = nc.dram_tensor("wp", [128, 6 * 128], dt.bfloat16,
                           kind="ExternalInput")
    wb_in = nc.dram_tensor("wb", [128, 2 * 128], dt.bfloat16,
                           kind="ExternalInput")
    ws_in = nc.dram_tensor("ws", [64, 2 * 128], dt.bfloat16,
                           kind="ExternalInput")
    id_in = nc.dram_tensor("ident", [128, 128], dt.bfloat16,
                           kind="ExternalInput")
    sc_in = nc.dram_tensor("scale2", [128, 1], dt.float32,
                           kind="ExternalInput")
    hout = nc.dram_tensor("hout", [T_STEPS, 64, OWN * W], dt.bfloat16,
                          kind="ExternalOutput")

    with TileContext(nc) as tc:
        with (
            tc.tile_pool(name="const", bufs=1) as cpool,
            tc.tile_pool(name="state", bufs=1) as spool,
            tc.tile_pool(name="xload", bufs=3) as xpool,
            tc.tile_pool(name="work", bufs=3) as wpool,
            tc.tile_pool(name="ps", bufs=2, space="PSUM") as psp,
        ):
            wp_sb = cpool.tile([128, 6 * 128], dt.bfloat16, tag="wp")
            wb_sb = cpool.tile([128, 2 * 128], dt.bfloat16, tag="wb")
            # ws lives on partitions 64-127 to match the hpad half of HB
            # (matmul requires lhsT and rhs at the same base partition)
            ws_sb = cpool.tile([128, 2 * 128], dt.bfloat16, tag="ws")
            id_sb = cpool.tile([128, 128], dt.bfloat16, tag="id")
            sc_sb = cpool.tile([128, 1], dt.float32, tag="sc")
            nc.sync.dma_start(out=wp_sb[:], in_=wp_in[:])
            nc.sync.dma_start(out=wb_sb[:], in_=wb_in[:])
            nc.sync.dma_start(out=ws_sb[64:128, :], in_=ws_in[:])
            nc.sync.dma_start(out=id_sb[:], in_=id_in[:])
            nc.sync.dma_start(out=sc_sb[:], in_=sc_in[:])

            # h buffers (ping-pong across steps) and cell state c.
            # bb: the dy=2 companion of hb -- partitions 64-127 mirror hpad,
            # partitions 0-63 hold hpad shifted LEFT one column, so one
            # K=128 matmul covers the (dy2,dx0)+(dy2,dx1) tap pair.
            hb = [
                spool.tile([128, HROWS, WP], dt.bfloat16, tag="hb0", name="hb0"),
                spool.tile([128, HROWS, WP], dt.bfloat16, tag="hb1", name="hb1"),
            ]
            bb = [
                spool.tile([128, HROWS, WP], dt.bfloat16, tag="bb0", name="bb0"),
                spool.tile([128, HROWS, WP], dt.bfloat16, tag="bb1", name="bb1"),
            ]
            cs = spool.tile([128, ROWS * W], dt.bfloat16, tag="cs")
            nc.vector.memset(hb[0][:], 0.0)
            nc.vector.memset(hb[1][:], 0.0)
            nc.vector.memset(bb[0][:], 0.0)
            nc.vector.memset(bb[1][:], 0.0)
            nc.vector.memset(cs[:], 0.0)

            def emit_tail(pend):
                """Lagged block tail: tc = tanh(c'); h' = sig_o*tc; copies."""
                p_hbw, p_bbw, y0, rows, Nb, sio, tch, last_t, step_last = pend
                cseg = cs[64:128, y0 * W: y0 * W + Nb]
                nc.scalar.activation(tch[64:128, :Nb], cseg, Act.Tanh,
                                     scale=1.0)
                nc.vector.tensor_tensor(
                    p_hbw[64:128, 1 + y0: 1 + y0 + rows, CL: CL + W],
                    sio[64:128, :Nb], tch[64:128, :Nb], Alu.mult)
                if not last_t:
                    # shifted copy for next step's K=128 pairs: lo[r] = hi[r+1]
                    nc.vector.tensor_copy(
                        p_hbw[0:64, y0: y0 + rows, :],
                        p_hbw[64:128, y0 + 1: y0 + 1 + rows, :])
                    # bb companion for the dy=2 (dx0,dx1) pair: partitions
                    # 0-63 = hpad shifted left 1 col, 64-127 = shifted left
                    # 2 cols. Flat contiguous copies; the wrap elements land
                    # in hpad's zero pad columns. Mid-step blocks go via
                    # SBUF->SBUF DMA (a full step of slack, off the compute
                    # engines); the step's LAST block is needed ~one block
                    # after the boundary, so DMA latency (~3.5us) would stall
                    # the PE -- do that one on DVE right behind the h-write.
                    base = (1 + y0) * WP
                    ln = rows * WP
                    hflat = p_hbw[64:128].rearrange("p a b -> p (a b)")
                    blo = p_bbw[0:64].rearrange("p a b -> p (a b)")
                    bhi = p_bbw[64:128].rearrange("p a b -> p (a b)")
                    nc.vector.tensor_copy(
                        blo[:, base: base + ln],
                        hflat[:, base + 1: base + 1 + ln])
                    nc.vector.tensor_copy(
                        bhi[:, base: base + ln],
                        hflat[:, base + 2: base + 2 + ln])

            pend = None      # one-block-lagged tail state
            pend_out = None  # deferred hout DMA: (t, hbw)
            for t in range(T_STEPS):
                R = 47 - t  # output rows this step
                hbr = hb[t % 2]
                hbw = hb[(t + 1) % 2]
                bbr = bb[t % 2]
                bbw = bb[(t + 1) % 2]

                xt = []
                for half in range(2):
                    xti = xpool.tile([128, ROWS * W], dt.bfloat16,
                                     tag=f"x{half}", name=f"x{half}")
                    nc.sync.dma_start(out=xti[:, : R * W],
                                      in_=x_in[t, half][:, : R * W])
                    xt.append(xti)

                nblk = (R + 15) // 16
                for bi in range(nblk):
                    y0 = bi * 16
                    rows = min(16, R - y0)
                    Nb = rows * W

                    tmp = [psp.tile([128, 1024], dt.float32, tag="tmp0",
                                    name="tmp0"),
                           psp.tile([128, 1024], dt.float32, tag="tmp1",
                                    name="tmp1")]
                    nsub = (rows + 7) // 8
                    # x injection first: h-independent PE work at block entry
                    for tau in range(2):
                        for sub in range(nsub):
                            yy = y0 + sub * 8
                            sr = min(8, rows - sub * 8)
                            n = sr * W
                            nc.tensor.matmul(
                                tmp[tau][:, sub * 512: sub * 512 + n],
                                lhsT=id_sb[:],
                                rhs=xt[tau][:, yy * W: yy * W + n],
                                start=True, stop=(t == 0))
                    if t > 0:
                        for tau in range(2):
                            for sub in range(nsub):
                                yy = y0 + sub * 8
                                sr = min(8, rows - sub * 8)
                                n = sr * W
                                out_ap = tmp[tau][:, sub * 512: sub * 512 + n]
                                for d in range(3):
                                    nc.tensor.matmul(
                                        out_ap,
                                        lhsT=wp_sb[:, (tau * 3 + d) * 128:
                                                   (tau * 3 + d + 1) * 128],
                                        rhs=hbr[:, yy: yy + sr,
                                                d + 1: d + 1 + W],
                                        start=False, stop=False)
                                # dy=2, dx2 single K=64 tap from hpad
                                nc.tensor.matmul(
                                    out_ap,
                                    lhsT=ws_sb[64:128, tau * 128:
                                               (tau + 1) * 128],
                                    rhs=hbr[64:128, yy + 2: yy + 2 + sr,
                                            3: 3 + W],
                                    start=False, stop=False)
                                # dy=2: (dx0,dx1) pair via the bb companion
                                # (last: its bb copies have the least slack)
                                nc.tensor.matmul(
                                    out_ap,
                                    lhsT=wb_sb[:, tau * 128:
                                               (tau + 1) * 128],
                                    rhs=bbr[:, yy + 2: yy + 2 + sr,
                                            0: W],
                                    start=False, stop=True)

                    # lagged tail from the previous block (possibly the last
                    # block of the previous step), then any deferred hout DMA
                    if pend is not None:
                        emit_tail(pend)
                        pend = None
                    if pend_out is not None:
                        pt, p_hbw = pend_out
                        nc.scalar.dma_start(
                            out=hout[pt],
                            in_=p_hbw[64:128, 1: 1 + OWN, CL: CL + W])
                        pend_out = None

                    # gates: tile0 = [g ; s_f=tanh(z/2)] (one Tanh act,
                    # per-partition scale), tile1 = [sig_i ; sig_o]
                    sgf = wpool.tile([128, 1024], dt.bfloat16, tag="sgf")
                    sio = wpool.tile([128, 1024], dt.bfloat16, tag="sio")
                    nc.scalar.activation(sgf[:, :Nb], tmp[0][:, :Nb],
                                         Act.Tanh, scale=sc_sb[:])
                    nc.scalar.activation(sio[:, :Nb], tmp[1][:, :Nb],
                                         Act.Sigmoid, scale=1.0)

                    # state update: w1 = 0.5*s_f + 0.5 (= f); u = f*c;
                    # v = sig_i*g (cross-half STT: inputs on 0-63, out on
                    # 64-127 -- engine output partition remap, inputs must
                    # match); c' = u + v
                    u = wpool.tile([128, 1024], dt.bfloat16, tag="u")
                    v = wpool.tile([128, 1024], dt.bfloat16, tag="v")
                    w1 = wpool.tile([128, 1024], dt.bfloat16, tag="w1")
                    tch = wpool.tile([128, 1024], dt.bfloat16, tag="tch")
                    cseg = cs[64:128, y0 * W: y0 * W + Nb]
                    nc.vector.tensor_scalar(
                        w1[64:128, :Nb], sgf[64:128, :Nb], 0.5, 0.5,
                        Alu.mult, Alu.add)
                    nc.vector.tensor_tensor(
                        u[64:128, :Nb], w1[64:128, :Nb], cseg, Alu.mult)
                    nc.vector.scalar_tensor_tensor(
                        v[64:128, :Nb], sio[0:64, :Nb], 1.0, sgf[0:64, :Nb],
                        Alu.mult, Alu.mult)
                    nc.vector.tensor_tensor(
                        cseg, u[64:128, :Nb], v[64:128, :Nb], Alu.add)

                    pend = (hbw, bbw, y0, rows, Nb, sio, tch, t == T_STEPS - 1,
                            bi == nblk - 1)

                pend_out = (t, hbw)

            # flush the final block's tail and the last output store
            emit_tail(pend)
            pt, p_hbw = pend_out
            nc.scalar.dma_start(out=hout[pt],
                                in_=p_hbw[64:128, 1: 1 + OWN, CL: CL + W])

    nc.finalize()
    return nc


def _prep_inputs(x, w_h2h):
    """Build per-core input maps. Cores: core = b*2 + half."""
    # gate order in PSUM tiles: tile0 = [g(0:64); f(64:128)],
    # tile1 = [i(0:64); o(64:128)]; reference channel order is [i, f, o, g]
    perm = np.concatenate([np.arange(192, 256), np.arange(64, 128),
                           np.arange(0, 64), np.arange(128, 192)])
    w_eff = w_h2h.astype(np.float32)[perm]  # [256, 64, 3, 3]

    def pack_w(weff):
        # wp lhsT rows: 0-63 multiply the row-shifted h copy (tap dy=1),
        # 64-127 multiply hpad (tap dy=0). wb covers dy=2 x (dx0, dx1) via
        # the bb companion (lo = hpad shifted left 1 col -> tap dx1);
        # ws covers the single remaining (dy2, dx2) tap.
        wp = np.zeros((128, 2, 3, 128), np.float32)
        wb = np.zeros((128, 2, 128), np.float32)
        ws = np.zeros((64, 2, 128), np.float32)
        for tau in range(2):
            blk = weff[tau * 128: (tau + 1) * 128]  # [128oc, 64ic, 3, 3]
            for d in range(3):
                wp[0:64, tau, d, :] = blk[:, :, 1, d].T
                wp[64:128, tau, d, :] = blk[:, :, 0, d].T
            wb[0:64, tau, :] = blk[:, :, 2, 0].T
            wb[64:128, tau, :] = blk[:, :, 2, 1].T
            ws[:, tau, :] = blk[:, :, 2, 2].T
        return (wp.reshape(128, 6 * 128).astype(bfloat16),
                wb.reshape(128, 2 * 128).astype(bfloat16),
                ws.reshape(64, 2 * 128).astype(bfloat16))

    wp_top, wb_top, ws_top = pack_w(w_eff)
    wp_bot, wb_bot, ws_bot = pack_w(w_eff[:, :, ::-1, :])
    ident = np.eye(128, dtype=np.float32).astype(bfloat16)
    scale2 = np.concatenate([np.ones((64, 1), np.float32),
                             np.full((64, 1), 0.5, np.float32)])

    xp = x[:, :, perm]  # [T, B, 256, H, W] permuted channels
    in_maps = []
    for b in range(B):
        for half in range(2):
            if half == 0:
                xs = xp[:, b, :, 0:ROWS, :]
            else:
                xs = xp[:, b, :, H - ROWS:, :][:, :, ::-1, :]
            xs = np.ascontiguousarray(xs).astype(bfloat16)
            xs = xs.reshape(T_STEPS, 2, 128, ROWS * W)
            in_maps.append({
                "x": xs,
                "wp": wp_top if half == 0 else wp_bot,
                "wb": wb_top if half == 0 else wb_bot,
                "ws": ws_top if half == 0 else ws_bot,
                "ident": ident,
                "scale2": scale2,
            })
    return in_maps


def kernel(x, w_h2h):
    from concourse import bass_utils

    if "nc" not in _CACHE:
        _CACHE["nc"] = _build_nc()
    nc = _CACHE["nc"]

    in_maps = _prep_inputs(np.asarray(x), np.asarray(w_h2h))
    res = bass_utils.run_bass_kernel_spmd(nc, in_maps,
                                          core_ids=list(range(8)))
    _CACHE["last_results"] = res

    out = np.zeros((T_STEPS, B, HIDDEN, H, W), np.float32)
    for b in range(B):
        for half in range(2):
            core = b * 2 + half
            hs = res.results[core]["hout"].astype(np.float32)
            hs = hs.reshape(T_STEPS, HIDDEN, OWN, W)
            if half == 0:
                out[:, b, :, 0:OWN, :] = hs
            else:
                out[:, b, :, OWN:, :] = hs[:, :, ::-1, :]
    return out


# revision 23
# speedup vs baseline: 1.2285x; 1.0583x over previous
"""ConvLSTM cell kernel for Trainium2 (8 NeuronCores).

Sharding: data-parallel over batch B=4 x spatial split of H=64 into 2 halves
(8 shards). The recurrence prevents sharding T. Each core computes its half
with a shrinking row margin (47-t rows at step t) so no cross-core
communication is ever needed: row validity shrinks by 1 per conv step, and
16 margin rows cover all 16 steps. Bottom halves are row-flipped on the host
(x rows flipped + conv kernel dy-flipped) so a single SPMD program serves
all 8 cores.

On-core layout:
  h lives in SBUF as [128, 49, 68] bf16 "HB": partitions 64-127 hold hpad
  (1 zero pad row on top, 2 zero pad cols left, 2 right), partitions 0-63
  hold the same data shifted down one row. A 3x3 conv then needs only 6
  matmul issues per 128-wide oc tile: 3 K=128 issues cover tap pairs
  (dy=1 on 0-63, dy=0 on 64-127) for dx=0..2, and 3 K=64 issues cover dy=2
  from the hpad half. x_t is added in PSUM with an identity matmul issued
  FIRST (start=True) so the step boundary has h-independent PE work.

Gate packing (host-side channel perm): tmp0 = [g(0:64); f(64:128)],
tmp1 = [i(0:64); o(64:128)]. One Tanh ACT with per-partition scale [1, 0.5]
gives [g ; s_f=tanh(z/2)]; one Sigmoid ACT gives [sig_i ; sig_o] (both
functions live in the `sigmoid_and_others` table set -> zero switches).
States c and h are carried UNSCALED on partitions 64-127.

State update (DVE; tensor_tensor runs in 2x packed-bf16 mode,
scalar_tensor_tensor is the only 1x op and the only cross-partition one):
  w1 = s_f + 1            (tensor_scalar, 4x)   = 2f
  u  = w1 * c             (TT, 2x)              = 2 f c
  v  = sig_i * g          (TT, 2x)
  c' = 0.5*u + v          (STT, cross-half add)
  tc = tanh(c')           (ACT)
  h' = sig_o * tc         (TT, 2x, into HB rows 64-127)
The tanh/h'/shifted-copy tail of each 16-row block is software-lagged by
one block (carried across the step boundary) so the Scalar and Vector FIFOs
never ping-pong head-of-line within a block, and the next step's matmuls
start with zero exposed serial tail.
"""

import sys

sys.path.insert(0, "/opt/trn_rl_repo")

import numpy as np
from ml_dtypes import bfloat16

HIDDEN = 64
T_STEPS = 16
B = 4
H = 64
W = 64
OC = 4 * HIDDEN  # 256
ROWS = 48        # per-core x rows (32 owned + 16 margin)
OWN = 32
WP = W + 4       # padded row width 68 (2 left, 2 right; keeps 4B alignment)
CL = 2           # left pad columns
HROWS = ROWS + 1  # hpad rows (1 zero row on top)

_CACHE = {}


def _build_nc():
    from concourse import bacc, mybir
    from concourse.tile import TileContext

    dt = mybir.dt
    Alu = mybir.AluOpType
    Act = mybir.ActivationFunctionType

    nc = bacc.Bacc(None, target_bir_lowering=False)

    x_in = nc.dram_tensor("x", [T_STEPS, 2, 128, ROWS * W], dt.bfloat16,
                          kind="ExternalInput")
    wp_in = nc.dram_tensor("wp", [128, 6 * 128], dt.bfloat16,
                           kind="ExternalInput")
    wb_in = nc.dram_tensor("wb", [128, 2 * 128], dt.bfloat16,
                           kind="ExternalInput")
    ws_in = nc.dram_tensor("ws", [64, 2 * 128], dt.bfloat16,
                           kind="ExternalInput")
    id_in = nc.dram_tensor("ident", [128, 128], dt.bfloat16,
                           kind="ExternalInput")
    sc_in = nc.dram_tensor("scale2", [128, 1], dt.float32,
                           kind="ExternalInput")
    hout = nc.dram_tensor("hout", [T_STEPS, 64, OWN * W], dt.bfloat16,
                          kind="ExternalOutput")

    with TileContext(nc) as tc:
        with (
            tc.tile_pool(name="const", bufs=1) as cpool,
            tc.tile_pool(name="state", bufs=1) as spool,
            tc.tile_pool(name="xload", bufs=3) as xpool,
            tc.tile_pool(name="work", bufs=3) as wpool,
            tc.tile_pool(name="ps", bufs=2, space="PSUM") as psp,
        ):
            wp_sb = cpool.tile([128, 6 * 128], dt.bfloat16, tag="wp")
            wb_sb = cpool.tile([128, 2 * 128], dt.bfloat16, tag="wb")
            # ws lives on partitions 64-127 to match the hpad half of HB
            # (matmul requires lhsT and rhs at the same base partition)
            ws_sb = cpool.tile([128, 2 * 128], dt.bfloat16, tag="ws")
            id_sb = cpool.tile([128, 128], dt.bfloat16, tag="id")
            sc_sb = cpool.tile([128, 1], dt.float32, tag="sc")
            nc.sync.dma_start(out=wp_sb[:], in_=wp_in[:])
            nc.sync.dma_start(out=wb_sb[:], in_=wb_in[:])
            nc.sync.dma_start(out=ws_sb[64:128, :], in_=ws_in[:])
            nc.sync.dma_start(out=id_sb[:], in_=id_in[:])
            nc.sync.dma_start(out=sc_sb[:], in_=sc_in[:])

            # h buffers (ping-pong across steps) and cell state c.
            # bb: the dy=2 companion of hb -- partitions 64-127 mirror hpad,
            # partitions 0-63 hold hpad shifted LEFT one column, so one
            # K=128 matmul covers the (dy2,dx0)+(dy2,dx1) tap pair.
            hb = [
                spool.tile([128, HROWS, WP], dt.bfloat16, tag="hb0", name="hb0"),
                spool.tile([128, HROWS, WP], dt.bfloat16, tag="hb1", name="hb1"),
            ]
            bb = [
                spool.tile([128, HROWS, WP], dt.bfloat16, tag="bb0", name="bb0"),
                spool.tile([128, HROWS, WP], dt.bfloat16, tag="bb1", name="bb1"),
            ]
            cs = spool.tile([128, ROWS * W], dt.bfloat16, tag="cs")
            nc.vector.memset(hb[0][:], 0.0)
            nc.vector.memset(hb[1][:], 0.0)
            nc.gpsimd.memset(bb[0][:], 0.0)
            nc.gpsimd.memset(bb[1][:], 0.0)
            nc.gpsimd.memset(cs[:], 0.0)

            def emit_tail(pend):
                """Lagged block tail: tc = tanh(c'); h' = sig_o*tc; copies."""
                p_hbw, p_bbw, y0, rows, Nb, sio, tch, last_t, step_last = pend
                # c' lives on partitions 0-63; ACT's output partition remap
                # carries tanh(c') over to 64-127 where sig_o lives
                cseg = cs[0:64, y0 * W: y0 * W + Nb]
                nc.scalar.activation(tch[64:128, :Nb], cseg, Act.Tanh,
                                     scale=1.0)
                nc.vector.tensor_tensor(
                    p_hbw[64:128, 1 + y0: 1 + y0 + rows, CL: CL + W],
                    sio[64:128, :Nb], tch[64:128, :Nb], Alu.mult)
                if not last_t:
                    # shifted copy for next step's K=128 pairs: lo[r] = hi[r+1]
                    nc.vector.tensor_copy(
                        p_hbw[0:64, y0: y0 + rows, :],
                        p_hbw[64:128, y0 + 1: y0 + 1 + rows, :])
                    # bb companion for the dy=2 (dx0,dx1) pair: partitions
                    # 0-63 = hpad shifted left 1 col, 64-127 = shifted left
                    # 2 cols. Flat contiguous copies; the wrap elements land
                    # in hpad's zero pad columns. Mid-step blocks go via
                    # SBUF->SBUF DMA (a full step of slack, off the compute
                    # engines); the step's LAST block is needed ~one block
                    # after the boundary, so DMA latency (~3.5us) would stall
                    # the PE -- do that one on DVE right behind the h-write.
                    base = (1 + y0) * WP
                    ln = rows * WP
                    hflat = p_hbw[64:128].rearrange("p a b -> p (a b)")
                    blo = p_bbw[0:64].rearrange("p a b -> p (a b)")
                    bhi = p_bbw[64:128].rearrange("p a b -> p (a b)")
                    nc.vector.tensor_copy(
                        blo[:, base: base + ln],
                        hflat[:, base + 1: base + 1 + ln])
                    nc.vector.tensor_copy(
                        bhi[:, base: base + ln],
                        hflat[:, base + 2: base + 2 + ln])

            pend = None      # one-block-lagged tail state
            pend_out = None  # deferred hout DMA: (t, hbw)
            for t in range(T_STEPS):
                R = 47 - t  # output rows this step
                hbr = hb[t % 2]
                hbw = hb[(t + 1) % 2]
                bbr = bb[t % 2]
                bbw = bb[(t + 1) % 2]

                xt = []
                for half in range(2):
                    xti = xpool.tile([128, ROWS * W], dt.bfloat16,
                                     tag=f"x{half}", name=f"x{half}")
                    if t == 0:
                        # split the cold-start load so the first matmuls can
                        # begin after the first 16-row chunk lands
                        for q0 in range(0, R, 16):
                            qn = min(16, R - q0) * W
                            nc.sync.dma_start(
                                out=xti[:, q0 * W: q0 * W + qn],
                                in_=x_in[t, half][:, q0 * W: q0 * W + qn])
                    else:
                        nc.sync.dma_start(out=xti[:, : R * W],
                                          in_=x_in[t, half][:, : R * W])
                    xt.append(xti)

                nblk = (R + 15) // 16
                for bi in range(nblk):
                    y0 = bi * 16
                    rows = min(16, R - y0)
                    Nb = rows * W

                    tmp = [psp.tile([128, 1024], dt.float32, tag="tmp0",
                                    name="tmp0"),
                           psp.tile([128, 1024], dt.float32, tag="tmp1",
                                    name="tmp1")]
                    nsub = (rows + 7) // 8
                    # x injection first: h-independent PE work at block entry
                    for tau in range(2):
                        for sub in range(nsub):
                            yy = y0 + sub * 8
                            sr = min(8, rows - sub * 8)
                            n = sr * W
                            nc.tensor.matmul(
                                tmp[tau][:, sub * 512: sub * 512 + n],
                                lhsT=id_sb[:],
                                rhs=xt[tau][:, yy * W: yy * W + n],
                                start=True, stop=(t == 0))
                    if t > 0:
                        for tau in range(2):
                            for sub in range(nsub):
                                yy = y0 + sub * 8
                                sr = min(8, rows - sub * 8)
                                n = sr * W
                                out_ap = tmp[tau][:, sub * 512: sub * 512 + n]
                                for d in range(3):
                                    nc.tensor.matmul(
                                        out_ap,
                                        lhsT=wp_sb[:, (tau * 3 + d) * 128:
                                                   (tau * 3 + d + 1) * 128],
                                        rhs=hbr[:, yy: yy + sr,
                                                d + 1: d + 1 + W],
                                        start=False, stop=False)
                                # dy=2, dx2 single K=64 tap from hpad
                                nc.tensor.matmul(
                                    out_ap,
                                    lhsT=ws_sb[64:128, tau * 128:
                                               (tau + 1) * 128],
                                    rhs=hbr[64:128, yy + 2: yy + 2 + sr,
                                            3: 3 + W],
                                    start=False, stop=False)
                                # dy=2: (dx0,dx1) pair via the bb companion
                                # (last: its bb copies have the least slack)
                                nc.tensor.matmul(
                                    out_ap,
                                    lhsT=wb_sb[:, tau * 128:
                                               (tau + 1) * 128],
                                    rhs=bbr[:, yy + 2: yy + 2 + sr,
                                            0: W],
                                    start=False, stop=True)

                    # lagged tail from the previous block (possibly the last
                    # block of the previous step), then any deferred hout DMA
                    if pend is not None:
                        emit_tail(pend)
                        pend = None
                    if pend_out is not None:
                        pt, p_hbw = pend_out
                        nc.scalar.dma_start(
                            out=hout[pt],
                            in_=p_hbw[64:128, 1: 1 + OWN, CL: CL + W])
                        pend_out = None

                    # gates: tile0 = [g ; s_f=tanh(z/2)] (one Tanh act,
                    # per-partition scale), tile1 = [sig_i ; sig_o]
                    sgf = wpool.tile([128, 1024], dt.bfloat16, tag="sgf")
                    sio = wpool.tile([128, 1024], dt.bfloat16, tag="sio")
                    nc.scalar.activation(sgf[:, :Nb], tmp[0][:, :Nb],
                                         Act.Tanh, scale=sc_sb[:])
                    nc.scalar.activation(sio[:, :Nb], tmp[1][:, :Nb],
                                         Act.Sigmoid, scale=1.0)

                    # state update, all on partitions 0-63 so every op is a
                    # 2x/4x-mode TT/TS (the tensor_scalar's output partition
                    # remap brings f = 0.5*s_f + 0.5 down from 64-127):
                    # u = f*c ; v = sig_i*g ; c' = u + v
                    u = wpool.tile([128, 1024], dt.bfloat16, tag="u")
                    v = wpool.tile([128, 1024], dt.bfloat16, tag="v")
                    w1 = wpool.tile([128, 1024], dt.bfloat16, tag="w1")
                    tch = wpool.tile([128, 1024], dt.bfloat16, tag="tch")
                    cseg = cs[0:64, y0 * W: y0 * W + Nb]
                    nc.vector.tensor_scalar(
                        w1[0:64, :Nb], sgf[64:128, :Nb], 0.5, 0.5,
                        Alu.mult, Alu.add)
                    nc.vector.tensor_tensor(
                        u[0:64, :Nb], w1[0:64, :Nb], cseg, Alu.mult)
                    nc.vector.tensor_tensor(
                        v[0:64, :Nb], sio[0:64, :Nb], sgf[0:64, :Nb],
                        Alu.mult)
                    nc.vector.tensor_tensor(
                        cseg, u[0:64, :Nb], v[0:64, :Nb], Alu.add)

                    pend = (hbw, bbw, y0, rows, Nb, sio, tch, t == T_STEPS - 1,
                            bi == nblk - 1)

                pend_out = (t, hbw)

            # flush the final block's tail and the last output store
            emit_tail(pend)
            pt, p_hbw = pend_out
            nc.scalar.dma_start(out=hout[pt],
                                in_=p_hbw[64:128, 1: 1 + OWN, CL: CL + W])

    nc.finalize()
    return nc


def _prep_inputs(x, w_h2h):
    """Build per-core input maps. Cores: core = b*2 + half."""
    # gate order in PSUM tiles: tile0 = [g(0:64); f(64:128)],
    # tile1 = [i(0:64); o(64:128)]; reference channel order is [i, f, o, g]
    perm = np.concatenate([np.arange(192, 256), np.arange(64, 128),
                           np.arange(0, 64), np.arange(128, 192)])
    w_eff = w_h2h.astype(np.float32)[perm]  # [256, 64, 3, 3]

    def pack_w(weff):
        # wp lhsT rows: 0-63 multiply the row-shifted h copy (tap dy=1),
        # 64-127 multiply hpad (tap dy=0). wb covers dy=2 x (dx0, dx1) via
        # the bb companion (lo = hpad shifted left 1 col -> tap dx1);
        # ws covers the single remaining (dy2, dx2) tap.
        wp = np.zeros((128, 2, 3, 128), np.float32)
        wb = np.zeros((128, 2, 128), np.float32)
        ws = np.zeros((64, 2, 128), np.float32)
        for tau in range(2):
            blk = weff[tau * 128: (tau + 1) * 128]  # [128oc, 64ic, 3, 3]
            for d in range(3):
                wp[0:64, tau, d, :] = blk[:, :, 1, d].T
                wp[64:128, tau, d, :] = blk[:, :, 0, d].T
            wb[0:64, tau, :] = blk[:, :, 2, 0].T
            wb[64:128, tau, :] = blk[:, :, 2, 1].T
            ws[:, tau, :] = blk[:, :, 2, 2].T
        return (wp.reshape(128, 6 * 128).astype(bfloat16),
                wb.reshape(128, 2 * 128).astype(bfloat16),
                ws.reshape(64, 2 * 128).astype(bfloat16))

    wp_top, wb_top, ws_top = pack_w(w_eff)
    wp_bot, wb_bot, ws_bot = pack_w(w_eff[:, :, ::-1, :])
    ident = np.eye(128, dtype=np.float32).astype(bfloat16)
    scale2 = np.concatenate([np.ones((64, 1), np.float32),
                             np.full((64, 1), 0.5, np.float32)])

    xp = x[:, :, perm]  # [T, B, 256, H, W] permuted channels
    in_maps = []
    for b in range(B):
        for half in range(2):
            if half == 0:
                xs = xp[:, b, :, 0:ROWS, :]
            else:
                xs = xp[:, b, :, H - ROWS:, :][:, :, ::-1, :]
            xs = np.ascontiguousarray(xs).astype(bfloat16)
            xs = xs.reshape(T_STEPS, 2, 128, ROWS * W)
            in_maps.append({
                "x": xs,
                "wp": wp_top if half == 0 else wp_bot,
                "wb": wb_top if half == 0 else wb_bot,
                "ws": ws_top if half == 0 else ws_bot,
                "ident": ident,
                "scale2": scale2,
            })
    return in_maps


def kernel(x, w_h2h):
    from concourse import bass_utils

    if "nc" not in _CACHE:
        _CACHE["nc"] = _build_nc()
    nc = _CACHE["nc"]

    in_maps = _prep_inputs(np.asarray(x), np.asarray(w_h2h))
    res = bass_utils.run_bass_kernel_spmd(nc, in_maps,
                                          core_ids=list(range(8)))
    _CACHE["last_results"] = res

    out = np.zeros((T_STEPS, B, HIDDEN, H, W), np.float32)
    for b in range(B):
        for half in range(2):
            core = b * 2 + half
            hs = res.results[core]["hout"].astype(np.float32)
            hs = hs.reshape(T_STEPS, HIDDEN, OWN, W)
            if half == 0:
                out[:, b, :, 0:OWN, :] = hs
            else:
                out[:, b, :, OWN:, :] = hs[:, :, ::-1, :]
    return out
